# revision 44
# baseline (speedup 1.0000x reference)
"""CTC loss kernel for Trainium2, data-parallel over batch across 8 NeuronCores.

Problem: pred [64, 64, 6736] f32 logits, gt [64, 16] int labels (< blank).
loss = mean_n( -log p_ctc(gt_n | log_softmax(pred_n)) / S ).

Per-core plan (8 examples/core), v2 — c-major layout:
  - Host stages the core's pred shard transposed to c-major bf16 with
    c_inner-major packing (class c at partition c%128, block c//128), so
    each SBUF partition owns a contiguous 53 KB HBM span for streaming
    AND every (c, n) pair owns a contiguous 64-element t-row, making the
    label gather ONE indirect DMA of 128 rows (vs 64 per-label gathers).
    bf16 staging halves HBM traffic to 6.9 MB (loss tolerance 2e-2;
    measured end-to-end error ~8e-6).  The gather offsets / skip masks /
    blank row ride in 3 extra packed rows per partition of the same
    tensor (one small DMA, no second input round trip).
  - Stream tiles are [128 = c-block, 512 = (n t)].  ACT exp (bf16 out,
    4-block pieces) feeds a ones-vector matmul chain on the otherwise
    idle PE that accumulates the per-(n,t) softmax denominators s[n,t]
    across the 53 c-blocks in one PSUM bank (cross-partition reduction).
  - Stream DMAs are HELD (dummy WAR readers) until the gather+regroup
    land: under full stream load each of the 128 random gather rows eats
    ~1µs of HBM queue latency, which would push the DP start from ~16µs
    to ~29µs.
  - CTC forward DP on DVE in the unnormalized prob domain, 3 ops/step
    (the serial t-loop is the kernel's critical path, ~200ns/op):
    the skip-transition mask is structural (blank-interleaved, no
    adjacent label repeats in the data) so the l-2 term is a strided
    16-wide add; a general masked 4-op fallback is built when repeats
    exist.  Rescaling cadence is host-derived from a per-(n,t) rowmax
    overflow bound; each rescale costs one reduce+reciprocal and the
    multiply is fused into the next step via scalar_tensor_tensor.
  - log p = ln(alpha[L-1]+alpha[L-2]) + sum ln C_k - sum_t ln s[n,t];
    per-example nll/S written to dram; host concatenates and means.

Hardware quirks handled here:
  - any instruction fits only ONE sync-wait command: _split_multi_waits
    legalizes every multi-wait instruction into a chain of single-wait
    no-ops on the same engine.
  - the ACT HWDGE ring hard-faults this device: all direct DMAs go on
    the SP ring, indirect/regroup + half the stream on gpsimd SWDGE.
  - tensor_tensor_reduce and AluOp.divide do not survive this walrus
    build's codegen; Tile reorders per-engine programs by readiness, so
    cross-path ordering is enforced with data deps (dummy corner
    writes), not emission order.
"""

import os

import numpy as np
import ml_dtypes

# Persistent XLA compilation cache: makes repeat kernel() calls skip the
# multi-minute neuronx-cc compile when the program is unchanged.
os.environ.setdefault("JAX_COMPILATION_CACHE_DIR", "/tmp/jax_comp_cache")

import concourse.bass as bass
import concourse.mybir as mybir
import concourse.tile as tile
from concourse.bass_utils import run_bass_kernel_spmd

F32 = mybir.dt.float32
BF16 = mybir.dt.bfloat16
I32 = mybir.dt.int32
AF = mybir.ActivationFunctionType
ALU = mybir.AluOpType

# Problem constants
N, T, C, S = 64, 64, 6736, 16
BLANK = C - 1
NCORES = 8
NL = N // NCORES            # examples per core
L = 2 * S + 1               # 33 extended labels
LP = L + 1                  # 34 (pad)
CPAD = 6784                 # 53 * 128
NCB = CPAD // 128           # 53 c-blocks
ROWS = CPAD * NL            # 54272 rows of T elements in the c-major view
XROWS = 384                 # +3 rows/partition: packed aux (2) + blank row (1)
ROWS2 = ROWS + XROWS
SFREE = NL * T              # 512 (n, t) columns per c-block
CHUNKS = [16, 16, 18, 3]    # stream-bound s-path: only the tiny last chunk
NSYNC_CHUNKS = 2            # matters (short post-stream exp+matmul tail)
# class c lives at partition p = c % 128, column block b = c // 128, so each
# partition owns a contiguous 53*512-element HBM span (sequential descriptors)
def crow(c):                # row index of (c, n=0) in the [ROWS, T] view
    return ((c % 128) * NCB + (c // 128)) * NL
# rescale cadence: alpha starts a window with sum 1 and grows by at most
# 3*e^max|pred| per step; a window of W steps stays under e^80 < f32 max
# worst-case when W*(ln3 + max|pred|) <= 80.  The host picks W from the data
# (W=12 for this problem's max|pred| of 5.42).
def renorm_ts(every):
    return [t for t in range(1, T) if t % every == 0]
PAD_VAL = -30000.0          # exp() underflows to exactly 0

AUXW = 1 + LP               # col 0: gather row idx; cols 1..34: mask (f32 bits)


def build_bass(structural: bool, renorm_every: int = 12):
    RENORM_TS = renorm_ts(renorm_every)
    NRE = len(RENORM_TS)
    nc = bass.Bass()
    predT = nc.dram_tensor("predT", [ROWS2, T], BF16, kind="ExternalInput")
    out = nc.dram_tensor("out", [NL, 1], F32, kind="ExternalOutput")

    with tile.TileContext(nc) as tc:
        with (
            tc.tile_pool(name="big", bufs=1) as bp,
            tc.tile_pool(name="small", bufs=1) as sp,
            tc.tile_pool(name="ps", bufs=1, space="PSUM") as pp,
        ):
            # ---------- one tiny load: packed aux + blank rows ----------
            # host packs per partition q: 2 rows of aux[q] bytes + 1 blank row
            ext_t = sp.tile([128, 3 * T], BF16)
            nc.sync.dma_start(out=ext_t[:], in_=predT[ROWS:ROWS2, :])
            idx_ap = ext_t[:, 0:2].bitcast(I32)          # aux col 0: row idx
            mask_ap = ext_t[0:NL, 2 : 2 + 2 * L].bitcast(F32)  # aux cols 1..33
            blk = ext_t[0:NL, 2 * T : 3 * T]

            # ---------- label gather: ONE indirect DMA, on idle HBM ----------
            pg = sp.tile([128, T], BF16)  # partition (n, j), row = t
            nc.gpsimd.indirect_dma_start(
                out=pg[:],
                out_offset=None,
                in_=predT[:],
                in_offset=bass.IndirectOffsetOnAxis(ap=idx_ap, axis=0),
            )
            # regroup [128=(n,j), t] -> [8, (j t)] (same linear order)
            pgr = sp.tile([NL, S * T], BF16)
            nc.gpsimd.dma_start(out=pgr[:], in_=pg[:])

            # ---------- stream DMAs, HELD until the gather lands ----------
            # Under full stream load each of the gather's 128 random HBM rows
            # costs ~1µs of queue latency, pushing the DP start out by ~15µs.
            # A dummy DVE read of pgr that writes each raw tile's corner makes
            # stream DMAs 1.. WAR/WAW-depend on the gather+regroup, so the
            # DP-critical 16 KB of random reads isn't buried under full
            # stream load (~1µs HBM queue latency per row).  Chunk 0 is
            # released immediately: one chunk's load leaves the gather enough
            # slots, and it feeds the ACT exp chain ~8µs earlier.
            # [ROWS, T] -> [128, (b n t)]: per-partition contiguous HBM span.
            view = predT[0:ROWS, :].rearrange("(p q) t -> p (q t)", p=128)
            raw_tiles = []
            for ci, ch in enumerate(CHUNKS):
                rt = bp.tile([128, ch * SFREE], BF16, tag=f"raw{ci}")
                nc.vector.tensor_copy(out=rt[0:1, 0:1], in_=pgr[0:1, 0:1])
                raw_tiles.append(rt)
            col = 0
            for ci, ch in enumerate(CHUNKS):
                eng = nc.sync if ci < NSYNC_CHUNKS else nc.gpsimd
                eng.dma_start(
                    out=raw_tiles[ci][:],
                    in_=view[:, col * SFREE : (col + ch) * SFREE],
                    max_dma_last_dim=2048,
                )
                col += ch

            # zero-AP biases keep bass from materializing const tensors
            # (saves the per-engine TENSOR_LOAD preamble)
            z8 = sp.tile([NL, 1], F32)
            nc.vector.memset(z8[:], 0.0)
            z128 = sp.tile([128, 1], F32)
            nc.vector.memset(z128[:], 0.0)

            # ---------- u3[n, t, l] built by ACT exp directly ----------
            u3 = sp.tile([NL, T * LP], F32)
            u3v = u3[:].rearrange("n (t l) -> n t l", l=LP)
            blk_b = bass.AP(
                blk.tensor, blk.offset, [blk.ap[0], [1, T], [0, S + 1]]
            )
            nc.scalar.activation(
                u3v[:, :, 0 : 2 * S + 1 : 2], blk_b, AF.Exp, bias=z8[:, 0:1]
            )
            # odd (label) entries split in two so the DP can start as soon as
            # the first 16 timesteps' u values exist
            pgr_v = pgr[:].rearrange("n (j t) -> n t j", t=T)
            TSPL = 8
            nc.scalar.activation(
                u3v[:, 0:TSPL, 1 : 2 * S : 2], pgr_v[:, 0:TSPL], AF.Exp,
                bias=z8[:, 0:1],
            )
            nc.scalar.activation(
                u3v[:, TSPL:T, 1 : 2 * S : 2], pgr_v[:, TSPL:T], AF.Exp,
                bias=z8[:, 0:1],
            )

            # ---------- stream exp (bf16 out), in <=4-block pieces ----------
            # finer exp granularity lets the PE matmul chain start one piece
            # (not one whole chunk) behind ACT, pulling the s-path tail in
            exp_tiles = []
            for ci, ch in enumerate(CHUNKS):
                et = bp.tile([128, ch * SFREE], BF16, tag=f"exp{ci}")
                for b0 in range(0, ch, 4):
                    bw = min(4, ch - b0) * SFREE
                    nc.scalar.activation(
                        et[:, b0 * SFREE : b0 * SFREE + bw],
                        raw_tiles[ci][:, b0 * SFREE : b0 * SFREE + bw],
                        AF.Exp,
                        bias=z128[:, 0:1],
                    )
                exp_tiles.append(et)

            # ---------- denominator: ones-matmul accumulation ----------
            ones = sp.tile([128, 1], BF16)
            nc.vector.memset(ones[:], 1.0)
            cinv = sp.tile([NL, 1], F32)
            nc.vector.memset(cinv[:], 1.0 / S)
            psum = pp.tile([1, SFREE], F32)
            ki = 0
            for ci, ch in enumerate(CHUNKS):
                for b in range(ch):
                    nc.tensor.matmul(
                        psum[:],
                        ones[:],
                        exp_tiles[ci][:, b * SFREE : (b + 1) * SFREE],
                        start=(ki == 0),
                        stop=(ki == NCB - 1),
                    )
                    ki += 1

            # ---------- CTC forward DP on DVE ----------
            buf = sp.tile([NL, LP + 1], F32)  # cols 0,1 guard zeros; 2.. alpha
            nc.vector.memset(buf[:], 0.0)
            x = sp.tile([NL, L], F32)
            x2 = sp.tile([NL, L], F32)
            rlog = sp.tile([NL, NRE + 1], F32)
            rinv = sp.tile([NL, 1], F32)
            a = buf[:, 2 : 2 + L]
            a1 = buf[:, 1 : 1 + L]
            a2 = buf[:, 0:L]
            mask_t = mask_ap

            # alpha init on ACT so DVE's first DP op isn't queued behind it
            nc.scalar.activation(buf[:, 2:4], u3v[:, 0, 0:2], AF.Copy)
            pending = False
            for t in range(1, T):
                ut = u3v[:, t, 0:L]
                nc.vector.tensor_add(out=x[:], in0=a, in1=a1)
                if structural:
                    # skip term lands on odd l only; a2|odd = buf cols 1,3..31
                    nc.vector.tensor_add(
                        out=x[:, 1:L:2], in0=x[:, 1:L:2], in1=buf[:, 1:32:2]
                    )
                else:
                    nc.vector.tensor_mul(out=x2[:], in0=a2, in1=mask_t)
                    nc.vector.tensor_add(out=x[:], in0=x[:], in1=x2[:])
                if pending:
                    # fused rescale from the renorm one step ago
                    nc.vector.scalar_tensor_tensor(
                        out=a,
                        in0=x[:],
                        scalar=rinv[:, 0:1],
                        in1=ut,
                        op0=ALU.mult,
                        op1=ALU.mult,
                    )
                    pending = False
                elif t in RENORM_TS:
                    k = RENORM_TS.index(t)
                    nc.vector.tensor_mul(out=a, in0=x[:], in1=ut)
                    nc.vector.tensor_reduce(
                        out=rlog[:, k : k + 1],
                        in_=a,
                        axis=mybir.AxisListType.X,
                        op=ALU.add,
                    )
                    nc.vector.reciprocal(out=rinv[:], in_=rlog[:, k : k + 1])
                    pending = True
                else:
                    nc.vector.tensor_mul(out=a, in0=x[:], in1=ut)
            # final forward prob: alpha[L-1] + alpha[L-2] = buf cols 34, 33
            nc.vector.tensor_add(
                out=rlog[:, NRE : NRE + 1],
                in0=buf[:, 33:34],
                in1=buf[:, 34:35],
            )

            # ---------- s path: ln + regroup + reduce ----------
            lns = sp.tile([1, SFREE], F32)
            nc.scalar.activation(lns[:], psum[:], AF.Ln, bias=z8[0:1, 0:1])
            srg = sp.tile([NL, T], F32)
            nc.sync.dma_start(out=srg[:], in_=lns[:])
            # reduce on ACT (idle by now) so the s-path never slots into
            # the DVE instruction stream mid-DP
            ssum = sp.tile([NL, 1], F32)
            srg2 = sp.tile([NL, T], F32)
            nc.scalar.activation(srg2[:], srg[:], AF.Copy, accum_out=ssum[:])

            # ---------- assembly ----------
            lnr = sp.tile([NL, NRE + 1], F32)
            nc.scalar.activation(lnr[:], rlog[:], AF.Ln, bias=z8[:, 0:1])
            rsum = sp.tile([NL, 1], F32)
            nc.vector.tensor_reduce(
                out=rsum[:], in_=lnr[:], axis=mybir.AxisListType.X, op=ALU.add
            )
            res2 = sp.tile([NL, 1], F32)
            nc.vector.scalar_tensor_tensor(
                out=res2[:],
                in0=ssum[:],
                scalar=rsum[:, 0:1],
                in1=cinv[:],
                op0=ALU.subtract,
                op1=ALU.mult,
            )
            nc.sync.dma_start(out=out[:], in_=res2[:])

    return nc


def _split_multi_waits(nc, maxw=1):
    """This compiler's codegen rejects >1 sync-wait command per instruction
    (setupSyncWait 'Too many sync wait commands').  Tile's kernel-tail drain
    aggregates one wait per live semaphore; split the excess into a chain of
    single-wait no-ops on the same engine right before the instruction."""
    for bb in nc.main_func.blocks:
        heavy = [
            (i, inst)
            for i, inst in enumerate(bb.instructions)
            if getattr(inst, "sync_info", None) is not None
            and inst.sync_info.on_wait
            and len(inst.sync_info.on_wait) > maxw
        ]
        for pos, inst in reversed(heavy):
            waits = list(inst.sync_info.on_wait)
            keep, extra = waits[:maxw], waits[maxw:]
            inst.sync_info = mybir.SyncInfo(
                on_wait=keep, on_update=list(inst.sync_info.on_update)
            )
            for j, w in enumerate(reversed(extra)):
                nop = mybir.InstNoOp(
                    name=f"{inst.name}-waitsplit-{j}",
                    ins=[],
                    outs=[],
                    sync_info=mybir.SyncInfo(on_wait=[w], on_update=[]),
                )
                nop.engine = inst.engine
                bb.instructions.insert(pos, nop)


def prepare_hw(nc):
    _split_multi_waits(nc)
    return nc


def make_core_inputs(pred_full, gt_full, core):
    nsl = slice(core * NL, (core + 1) * NL)
    predc = np.asarray(pred_full[nsl], dtype=np.float32)  # [8, 64, 6736]
    gtc = np.asarray(gt_full[nsl]).astype(np.int64)

    # c-major bf16, c_inner-major packing: [128, NCB, NL, T] -> [ROWS, T]
    # class c sits at [c % 128, c // 128]; partition rows are contiguous HBM
    pt = np.full((128, NCB, NL, T), PAD_VAL, dtype=np.float32)
    cm = np.arange(C)
    pt[cm % 128, cm // 128] = predc.transpose(2, 0, 1)
    predT = pt.reshape(ROWS, T).astype(ml_dtypes.bfloat16)

    aux = np.zeros((128, 64), np.int32)  # 64 i32 = 2 bf16 rows of 64
    # gather row index for partition q = (n, j)
    p_n = np.arange(128) // S
    p_j = np.arange(128) % S
    cq = gtc[p_n, p_j]
    aux[:, 0] = ((cq % 128) * NCB + (cq // 128)) * NL + p_n

    m = np.zeros((NL, LP), np.float32)
    m[:, 1] = 1.0
    for j in range(1, S):
        m[:, 2 * j + 1] = (gtc[:, j] != gtc[:, j - 1]).astype(np.float32)
    aux[0:NL, 1 : 1 + LP] = m.view(np.int32)

    # pack per partition q: rows 3q,3q+1 = aux[q] raw bytes, 3q+2 = blank row
    ext = np.zeros((128, 3, T), np.uint16)
    ext[:, 0:2, :] = aux.view(np.uint16).reshape(128, 2, T)
    ext[0:NL, 2, :] = predT[crow(BLANK) : crow(BLANK) + NL].view(np.uint16)
    full = np.concatenate(
        [predT.view(np.uint16), ext.reshape(XROWS, T)], axis=0
    ).view(ml_dtypes.bfloat16)
    return {"predT": full}


_NC_CACHE = {}


def kernel(pred, gt):
    gt64 = np.asarray(gt).astype(np.int64)
    structural = not bool((gt64[:, 1:] == gt64[:, :-1]).any())
    # widest guaranteed-overflow-safe rescale window: per step the alpha sum
    # grows by at most 3*exp(rowmax[n,t]); find the largest uniform cadence
    # whose worst window (by actual per-(n,t) rowmax) stays under e^80
    cost = np.log(3.0) + np.maximum(
        np.asarray(pred, dtype=np.float32).max(axis=2), 0.0
    )  # [N, T]
    re = 4
    for cand in range(16, 4, -1):
        ts = [0] + [t for t in range(1, T) if t % cand == 0] + [T]
        wmax = max(
            cost[:, a:b].sum(axis=1).max() for a, b in zip(ts[:-1], ts[1:])
        )
        if wmax <= 80.0:
            re = cand
            break
    in_maps = [make_core_inputs(pred, gt, c) for c in range(NCORES)]
    key = f"nc_{structural}_{re}"
    if key not in _NC_CACHE:
        _NC_CACHE[key] = prepare_hw(build_bass(structural, re))
    nc = _NC_CACHE[key]
    res = run_bass_kernel_spmd(nc, in_maps, core_ids=list(range(NCORES)))
    _NC_CACHE["last_results"] = res
    vals = np.concatenate([r["out"][:, 0] for r in res.results])
    return np.array(vals.mean(), dtype=np.float32)


if __name__ == "__main__":
    rng = np.random.default_rng(0)
    pred = rng.standard_normal((N, T, C), dtype=np.float32)
    gt = rng.integers(0, BLANK, size=(N, S)).astype(np.int32)
    print(kernel(pred=pred, gt=gt))


# revision 46
# speedup vs baseline: 1.1959x; 1.1959x over previous
"""CTC loss kernel for Trainium2, data-parallel over batch across 8 NeuronCores.

Problem: pred [64, 64, 6736] f32 logits, gt [64, 16] int labels (< blank).
loss = mean_n( -log p_ctc(gt_n | log_softmax(pred_n)) / S ).

Per-core plan (8 examples/core), v2 — c-major layout:
  - Host stages the core's pred shard transposed to c-major bf16 with
    c_inner-major packing (class c at partition c%128, block c//128), so
    each SBUF partition owns a contiguous 53 KB HBM span for streaming
    AND every (c, n) pair owns a contiguous 64-element t-row, making the
    label gather ONE indirect DMA of 128 rows (vs 64 per-label gathers).
    bf16 staging halves HBM traffic to 6.9 MB (loss tolerance 2e-2;
    measured end-to-end error ~8e-6).  The gather offsets / skip masks /
    blank row ride in 3 extra packed rows per partition of the same
    tensor (one small DMA, no second input round trip).
  - Stream tiles are [128 = c-block, 512 = (n t)].  ACT exp (bf16 out,
    4-block pieces) feeds a ones-vector matmul chain on the otherwise
    idle PE that accumulates the per-(n,t) softmax denominators s[n,t]
    across the 53 c-blocks in one PSUM bank (cross-partition reduction).
  - Stream DMAs are HELD (dummy WAR readers) until the gather+regroup
    land: under full stream load each of the 128 random gather rows eats
    ~1µs of HBM queue latency, which would push the DP start from ~16µs
    to ~29µs.
  - CTC forward DP on DVE in the unnormalized prob domain, 3 ops/step
    (the serial t-loop is the kernel's critical path, ~200ns/op):
    the skip-transition mask is structural (blank-interleaved, no
    adjacent label repeats in the data) so the l-2 term is a strided
    16-wide add; a general masked 4-op fallback is built when repeats
    exist.  Rescaling cadence is host-derived from a per-(n,t) rowmax
    overflow bound; each rescale costs one reduce+reciprocal and the
    multiply is fused into the next step via scalar_tensor_tensor.
  - log p = ln(alpha[L-1]+alpha[L-2]) + sum ln C_k - sum_t ln s[n,t];
    per-example nll/S written to dram; host concatenates and means.

Hardware quirks handled here:
  - any instruction fits only ONE sync-wait command: _split_multi_waits
    legalizes every multi-wait instruction into a chain of single-wait
    no-ops on the same engine.
  - the ACT HWDGE ring hard-faults this device: all direct DMAs go on
    the SP ring, indirect/regroup + half the stream on gpsimd SWDGE.
  - tensor_tensor_reduce and AluOp.divide do not survive this walrus
    build's codegen; Tile reorders per-engine programs by readiness, so
    cross-path ordering is enforced with data deps (dummy corner
    writes), not emission order.
"""

import os

import numpy as np
import ml_dtypes

# Persistent XLA compilation cache: makes repeat kernel() calls skip the
# multi-minute neuronx-cc compile when the program is unchanged.
os.environ.setdefault("JAX_COMPILATION_CACHE_DIR", "/tmp/jax_comp_cache")

import concourse.bass as bass
import concourse.mybir as mybir
import concourse.tile as tile
from concourse.bass_utils import run_bass_kernel_spmd

F32 = mybir.dt.float32
BF16 = mybir.dt.bfloat16
I32 = mybir.dt.int32
AF = mybir.ActivationFunctionType
ALU = mybir.AluOpType

# Problem constants
N, T, C, S = 64, 64, 6736, 16
BLANK = C - 1
NCORES = 8
NL = N // NCORES            # examples per core
L = 2 * S + 1               # 33 extended labels
LP = L + 1                  # 34 (pad)
CPAD = 6784                 # 53 * 128
NCB = CPAD // 128           # 53 c-blocks
ROWS = CPAD * NL            # 54272 rows of T elements in the c-major view
XROWS = 384                 # +3 rows/partition: packed aux (2) + blank row (1)
ROWS2 = ROWS + XROWS
SFREE = NL * T              # 512 (n, t) columns per c-block
CHUNKS = [16, 16, 18, 3]    # stream-bound s-path: only the tiny last chunk
NSYNC_CHUNKS = 2            # matters (short post-stream exp+matmul tail)
# class c lives at partition p = c % 128, column block b = c // 128, so each
# partition owns a contiguous 53*512-element HBM span (sequential descriptors)
def crow(c):                # row index of (c, n=0) in the [ROWS, T] view
    return ((c % 128) * NCB + (c // 128)) * NL
# rescale cadence: alpha starts a window with sum 1 and grows by at most
# 3*e^max|pred| per step; a window of W steps stays under e^80 < f32 max
# worst-case when W*(ln3 + max|pred|) <= 80.  The host picks W from the data
# (W=12 for this problem's max|pred| of 5.42).
def renorm_ts(every):
    return [t for t in range(1, T) if t % every == 0]
PAD_VAL = -30000.0          # exp() underflows to exactly 0

AUXW = 1 + LP               # col 0: gather row idx; cols 1..34: mask (f32 bits)


def build_bass(structural: bool, renorm_every: int = 12):
    RENORM_TS = renorm_ts(renorm_every)
    NRE = len(RENORM_TS)
    nc = bass.Bass()
    predT = nc.dram_tensor("predT", [ROWS2, T], BF16, kind="ExternalInput")
    out = nc.dram_tensor("out", [NL, 1], F32, kind="ExternalOutput")

    with tile.TileContext(nc) as tc:
        with (
            tc.tile_pool(name="big", bufs=1) as bp,
            tc.tile_pool(name="small", bufs=1) as sp,
            tc.tile_pool(name="ps", bufs=1, space="PSUM") as pp,
        ):
            # ---------- one tiny load: packed aux + blank rows ----------
            # host packs per partition q: 2 rows of aux[q] bytes + 1 blank row
            ext_t = sp.tile([128, 3 * T], BF16)
            nc.sync.dma_start(out=ext_t[:], in_=predT[ROWS:ROWS2, :])
            idx_ap = ext_t[:, 0:2].bitcast(I32)          # aux col 0: row idx
            mask_ap = ext_t[0:NL, 2 : 2 + 2 * L].bitcast(F32)  # aux cols 1..33
            blk = ext_t[0:NL, 2 * T : 3 * T]

            # ---------- label gather: ONE indirect DMA, on idle HBM ----------
            pg = sp.tile([128, T], BF16)  # partition (n, j), row = t
            nc.gpsimd.indirect_dma_start(
                out=pg[:],
                out_offset=None,
                in_=predT[:],
                in_offset=bass.IndirectOffsetOnAxis(ap=idx_ap, axis=0),
            )
            # regroup [128=(n,j), t] -> [8, (j t)] (same linear order)
            pgr = sp.tile([NL, S * T], BF16)
            nc.gpsimd.dma_start(out=pgr[:], in_=pg[:])

            # ---------- stream DMAs, HELD until the gather lands ----------
            # Under full stream load each of the gather's 128 random HBM rows
            # costs ~1µs of queue latency, pushing the DP start out by ~15µs.
            # A dummy DVE read of pgr that writes each raw tile's corner makes
            # every stream DMA WAR/WAW-depend on the gather+regroup, so the
            # DP-critical 16 KB of random reads isn't buried under full
            # stream load (~1µs HBM queue latency per row).
            # [ROWS, T] -> [128, (b n t)]: per-partition contiguous HBM span.
            view = predT[0:ROWS, :].rearrange("(p q) t -> p (q t)", p=128)
            raw_tiles = []
            for ci, ch in enumerate(CHUNKS):
                rt = bp.tile([128, ch * SFREE], BF16, tag=f"raw{ci}")
                nc.vector.tensor_copy(out=rt[0:1, 0:1], in_=pgr[0:1, 0:1])
                raw_tiles.append(rt)
            col = 0
            for ci, ch in enumerate(CHUNKS):
                eng = nc.sync if ci < NSYNC_CHUNKS else nc.gpsimd
                eng.dma_start(
                    out=raw_tiles[ci][:],
                    in_=view[:, col * SFREE : (col + ch) * SFREE],
                    max_dma_last_dim=2048,
                )
                col += ch

            # zero-AP biases keep bass from materializing const tensors
            # (saves the per-engine TENSOR_LOAD preamble)
            z8 = sp.tile([NL, 1], F32)
            nc.vector.memset(z8[:], 0.0)
            z128 = sp.tile([128, 1], F32)
            nc.vector.memset(z128[:], 0.0)

            # ---------- u3[n, t, l] built by ACT exp directly ----------
            u3 = sp.tile([NL, T * LP], F32)
            u3v = u3[:].rearrange("n (t l) -> n t l", l=LP)
            blk_b = bass.AP(
                blk.tensor, blk.offset, [blk.ap[0], [1, T], [0, S + 1]]
            )
            nc.scalar.activation(
                u3v[:, :, 0 : 2 * S + 1 : 2], blk_b, AF.Exp, bias=z8[:, 0:1]
            )
            # odd (label) entries split in two so the DP can start as soon as
            # the first 16 timesteps' u values exist
            pgr_v = pgr[:].rearrange("n (j t) -> n t j", t=T)
            TSPL = 8
            nc.scalar.activation(
                u3v[:, 0:TSPL, 1 : 2 * S : 2], pgr_v[:, 0:TSPL], AF.Exp,
                bias=z8[:, 0:1],
            )
            nc.scalar.activation(
                u3v[:, TSPL:T, 1 : 2 * S : 2], pgr_v[:, TSPL:T], AF.Exp,
                bias=z8[:, 0:1],
            )

            # ---------- stream exp (bf16 out), in <=4-block pieces ----------
            # finer exp granularity lets the PE matmul chain start one piece
            # (not one whole chunk) behind ACT, pulling the s-path tail in
            exp_tiles = []
            for ci, ch in enumerate(CHUNKS):
                et = bp.tile([128, ch * SFREE], BF16, tag=f"exp{ci}")
                for b0 in range(0, ch, 4):
                    bw = min(4, ch - b0) * SFREE
                    nc.scalar.activation(
                        et[:, b0 * SFREE : b0 * SFREE + bw],
                        raw_tiles[ci][:, b0 * SFREE : b0 * SFREE + bw],
                        AF.Exp,
                        bias=z128[:, 0:1],
                    )
                exp_tiles.append(et)

            # ---------- denominator: ones-matmul accumulation ----------
            ones = sp.tile([128, 1], BF16)
            nc.vector.memset(ones[:], 1.0)
            cinv = sp.tile([NL, 1], F32)
            nc.vector.memset(cinv[:], 1.0 / S)
            psum = pp.tile([1, SFREE], F32)
            ki = 0
            for ci, ch in enumerate(CHUNKS):
                for b in range(ch):
                    nc.tensor.matmul(
                        psum[:],
                        ones[:],
                        exp_tiles[ci][:, b * SFREE : (b + 1) * SFREE],
                        start=(ki == 0),
                        stop=(ki == NCB - 1),
                    )
                    ki += 1

            # ---------- CTC forward DP on DVE ----------
            buf = sp.tile([NL, LP + 1], F32)  # cols 0,1 guard zeros; 2.. alpha
            nc.vector.memset(buf[:], 0.0)
            x = sp.tile([NL, L], F32)
            x2 = sp.tile([NL, L], F32)
            rlog = sp.tile([NL, NRE + 1], F32)
            rinv = sp.tile([NL, 1], F32)
            a = buf[:, 2 : 2 + L]
            a1 = buf[:, 1 : 1 + L]
            a2 = buf[:, 0:L]
            mask_t = mask_ap

            nc.vector.tensor_copy(out=buf[:, 2:4], in_=u3v[:, 0, 0:2])
            pending = False
            for t in range(1, T):
                ut = u3v[:, t, 0:L]
                nc.vector.tensor_add(out=x[:], in0=a, in1=a1)
                if structural:
                    # skip term lands on odd l only; a2|odd = buf cols 1,3..31
                    nc.vector.tensor_add(
                        out=x[:, 1:L:2], in0=x[:, 1:L:2], in1=buf[:, 1:32:2]
                    )
                else:
                    nc.vector.tensor_mul(out=x2[:], in0=a2, in1=mask_t)
                    nc.vector.tensor_add(out=x[:], in0=x[:], in1=x2[:])
                if pending:
                    # fused rescale from the renorm one step ago
                    nc.vector.scalar_tensor_tensor(
                        out=a,
                        in0=x[:],
                        scalar=rinv[:, 0:1],
                        in1=ut,
                        op0=ALU.mult,
                        op1=ALU.mult,
                    )
                    pending = False
                elif t in RENORM_TS:
                    k = RENORM_TS.index(t)
                    nc.vector.tensor_mul(out=a, in0=x[:], in1=ut)
                    nc.vector.tensor_reduce(
                        out=rlog[:, k : k + 1],
                        in_=a,
                        axis=mybir.AxisListType.X,
                        op=ALU.add,
                    )
                    nc.vector.reciprocal(out=rinv[:], in_=rlog[:, k : k + 1])
                    pending = True
                else:
                    nc.vector.tensor_mul(out=a, in0=x[:], in1=ut)
            # final forward prob: alpha[L-1] + alpha[L-2] = buf cols 34, 33
            nc.vector.tensor_add(
                out=rlog[:, NRE : NRE + 1],
                in0=buf[:, 33:34],
                in1=buf[:, 34:35],
            )

            # ---------- s path: ln + regroup + reduce ----------
            lns = sp.tile([1, SFREE], F32)
            nc.scalar.activation(lns[:], psum[:], AF.Ln, bias=z8[0:1, 0:1])
            srg = sp.tile([NL, T], F32)
            nc.sync.dma_start(out=srg[:], in_=lns[:])
            # reduce on ACT (idle by now) so the s-path never slots into
            # the DVE instruction stream mid-DP
            ssum = sp.tile([NL, 1], F32)
            srg2 = sp.tile([NL, T], F32)
            nc.scalar.activation(srg2[:], srg[:], AF.Copy, accum_out=ssum[:])

            # ---------- assembly ----------
            lnr = sp.tile([NL, NRE + 1], F32)
            nc.scalar.activation(lnr[:], rlog[:], AF.Ln, bias=z8[:, 0:1])
            rsum = sp.tile([NL, 1], F32)
            nc.vector.tensor_reduce(
                out=rsum[:], in_=lnr[:], axis=mybir.AxisListType.X, op=ALU.add
            )
            res2 = sp.tile([NL, 1], F32)
            nc.vector.scalar_tensor_tensor(
                out=res2[:],
                in0=ssum[:],
                scalar=rsum[:, 0:1],
                in1=cinv[:],
                op0=ALU.subtract,
                op1=ALU.mult,
            )
            nc.sync.dma_start(out=out[:], in_=res2[:])

    return nc


def _split_multi_waits(nc, maxw=1):
    """This compiler's codegen rejects >1 sync-wait command per instruction
    (setupSyncWait 'Too many sync wait commands').  Tile's kernel-tail drain
    aggregates one wait per live semaphore; split the excess into a chain of
    single-wait no-ops on the same engine right before the instruction."""
    for bb in nc.main_func.blocks:
        heavy = [
            (i, inst)
            for i, inst in enumerate(bb.instructions)
            if getattr(inst, "sync_info", None) is not None
            and inst.sync_info.on_wait
            and len(inst.sync_info.on_wait) > maxw
        ]
        for pos, inst in reversed(heavy):
            waits = list(inst.sync_info.on_wait)
            keep, extra = waits[:maxw], waits[maxw:]
            inst.sync_info = mybir.SyncInfo(
                on_wait=keep, on_update=list(inst.sync_info.on_update)
            )
            for j, w in enumerate(reversed(extra)):
                nop = mybir.InstNoOp(
                    name=f"{inst.name}-waitsplit-{j}",
                    ins=[],
                    outs=[],
                    sync_info=mybir.SyncInfo(on_wait=[w], on_update=[]),
                )
                nop.engine = inst.engine
                bb.instructions.insert(pos, nop)


def prepare_hw(nc):
    _split_multi_waits(nc)
    return nc


def make_core_inputs(pred_full, gt_full, core):
    nsl = slice(core * NL, (core + 1) * NL)
    predc = np.asarray(pred_full[nsl], dtype=np.float32)  # [8, 64, 6736]
    gtc = np.asarray(gt_full[nsl]).astype(np.int64)

    # c-major bf16, c_inner-major packing: [128, NCB, NL, T] -> [ROWS, T]
    # class c sits at [c % 128, c // 128]; partition rows are contiguous HBM
    pt = np.full((128, NCB, NL, T), PAD_VAL, dtype=np.float32)
    cm = np.arange(C)
    pt[cm % 128, cm // 128] = predc.transpose(2, 0, 1)
    predT = pt.reshape(ROWS, T).astype(ml_dtypes.bfloat16)

    aux = np.zeros((128, 64), np.int32)  # 64 i32 = 2 bf16 rows of 64
    # gather row index for partition q = (n, j)
    p_n = np.arange(128) // S
    p_j = np.arange(128) % S
    cq = gtc[p_n, p_j]
    aux[:, 0] = ((cq % 128) * NCB + (cq // 128)) * NL + p_n

    m = np.zeros((NL, LP), np.float32)
    m[:, 1] = 1.0
    for j in range(1, S):
        m[:, 2 * j + 1] = (gtc[:, j] != gtc[:, j - 1]).astype(np.float32)
    aux[0:NL, 1 : 1 + LP] = m.view(np.int32)

    # pack per partition q: rows 3q,3q+1 = aux[q] raw bytes, 3q+2 = blank row
    ext = np.zeros((128, 3, T), np.uint16)
    ext[:, 0:2, :] = aux.view(np.uint16).reshape(128, 2, T)
    ext[0:NL, 2, :] = predT[crow(BLANK) : crow(BLANK) + NL].view(np.uint16)
    full = np.concatenate(
        [predT.view(np.uint16), ext.reshape(XROWS, T)], axis=0
    ).view(ml_dtypes.bfloat16)
    return {"predT": full}


_NC_CACHE = {}


def kernel(pred, gt):
    gt64 = np.asarray(gt).astype(np.int64)
    structural = not bool((gt64[:, 1:] == gt64[:, :-1]).any())
    # widest guaranteed-overflow-safe rescale window: per step the alpha sum
    # grows by at most 3*exp(rowmax[n,t]); find the largest uniform cadence
    # whose worst window (by actual per-(n,t) rowmax) stays under e^80
    cost = np.log(3.0) + np.maximum(
        np.asarray(pred, dtype=np.float32).max(axis=2), 0.0
    )  # [N, T]
    re = 4
    for cand in range(16, 4, -1):
        ts = [0] + [t for t in range(1, T) if t % cand == 0] + [T]
        wmax = max(
            cost[:, a:b].sum(axis=1).max() for a, b in zip(ts[:-1], ts[1:])
        )
        if wmax <= 80.0:
            re = cand
            break
    in_maps = [make_core_inputs(pred, gt, c) for c in range(NCORES)]
    key = f"nc_{structural}_{re}"
    if key not in _NC_CACHE:
        _NC_CACHE[key] = prepare_hw(build_bass(structural, re))
    nc = _NC_CACHE[key]
    res = run_bass_kernel_spmd(nc, in_maps, core_ids=list(range(NCORES)))
    _NC_CACHE["last_results"] = res
    vals = np.concatenate([r["out"][:, 0] for r in res.results])
    return np.array(vals.mean(), dtype=np.float32)


if __name__ == "__main__":
    rng = np.random.default_rng(0)
    pred = rng.standard_normal((N, T, C), dtype=np.float32)
    gt = rng.integers(0, BLANK, size=(N, S)).astype(np.int32)
    print(kernel(pred=pred, gt=gt))
